# revision 21
# baseline (speedup 1.0000x reference)
"""Trainium2 Bass kernel for CurvSelfAttention (B=2, S=2048, E=1024, H=16).

Sharding: 8 cores = 2 batches x 4 head-quads. Core c handles batch c//4 and
heads [4*(c%4), 4*(c%4)+4). Attention is head-independent, so there are no
collectives; each core gets its batch's hidden states and its heads' weight
row-slices, and returns a [S, 256] slice of the output.

v3 design, from on-silicon microbenchmarks:
  - ScalarE exp is the hard floor: 128 x [128,1024] exp tiles at ~1.15us
    each (1.2 GHz fixed). Everything else is scheduled around keeping
    ScalarE 100% busy on exp.
  - exp OUTPUT dtype f32r is the fast ACT path (1115ns vs 1337 f32 /
    1403 bf16 / 1521 fp16), so ets and the ctx matmuls are f32r
    (32/16-bit matmul mixing is not supported).
  - fp16 matmuls run at full rate (215ns/MM N=512) with near-exact
    products (fp16 mult -> f32 accumulate), so projections and scores
    use fp16 operands; transposes ride the DMA XBAR (2-byte only).
  - PE HAM re-throttles to 1.2 GHz after ~3.4us of idle. The attention
    loop therefore interleaves at t-tile granularity: scores(sec, t) +
    ctx(sec-1, t) keeps the PE active in every HAM window, and V/Q/s
    projection work is spread into the early sections.

Per-core program:
  A. hid/weights -> fp16 (DVE/GpSimd) -> [do, j, m] transposed layouts
     via XBAR (dma_start_transpose).
  B. K projection first (scores need full KT), then s/Q for qblk 0.
  C. 8 sections (qblk, g): 16 paired half-array score matmuls (two heads
     on partition halves 0/64) -> exp (scale=1/8, no row max) -> f32r ets;
     ctx for the previous section accumulates VA (f32r, ones column for
     the softmax denominator) over t interleaved with the scores; V and
     remaining s/Q projections fill sections 0-1; psum ctx -> fp16 ->
     XBAR transpose -> divide by denominator on DVE -> f32 out.
"""

import numpy as np

import concourse.bass as bass
import concourse.mybir as mybir
import concourse.tile as tile
from concourse import bacc, bass_utils

S = 2048
E = 1024
HL = 4          # heads per core
DH = 64         # head dim
NG = 2          # head groups per core (2 heads each -> 128 partitions)
EJ = E // 128   # 8 contraction tiles
ST = S // 128   # 16 sequence tiles
QB = 512        # projection / attention q block
NSEC = (S // QB) * NG
F32 = mybir.dt.float32
F32R = mybir.dt.float32r
FP16 = mybir.dt.float16


def build_program(nc, reps=1, stages="all"):
    hid = nc.dram_tensor("hid", [S, E], F32, kind="ExternalInput")
    wq = nc.dram_tensor("wq", [HL * DH, E], F32, kind="ExternalInput")
    wk = nc.dram_tensor("wk", [HL * DH, E], F32, kind="ExternalInput")
    wv = nc.dram_tensor("wv", [HL * DH, E], F32, kind="ExternalInput")
    ws = nc.dram_tensor("ws", [64, E], F32, kind="ExternalInput")
    bq = nc.dram_tensor("bq", [HL * DH], F32, kind="ExternalInput")
    bk = nc.dram_tensor("bk", [HL * DH], F32, kind="ExternalInput")
    bv = nc.dram_tensor("bv", [HL * DH], F32, kind="ExternalInput")
    bs = nc.dram_tensor("bs", [64], F32, kind="ExternalInput")
    out = nc.dram_tensor("out", [S, HL * DH], F32, kind="ExternalOutput")

    AF = mybir.ActivationFunctionType
    ALU = mybir.AluOpType

    with tile.TileContext(nc) as tc:
        def emit(pfx):
            with (
                tc.tile_pool(name=pfx + "const", bufs=1) as cpool,
                tc.tile_pool(name=pfx + "qkv", bufs=1) as qkv,
                tc.tile_pool(name=pfx + "raw", bufs=2) as raw,
                tc.tile_pool(name=pfx + "hbf", bufs=3) as hbf,
                tc.tile_pool(name=pfx + "outp", bufs=2) as outp,
                tc.tile_pool(name=pfx + "expT", bufs=17) as expp,
                tc.tile_pool(name=pfx + "ctxsb", bufs=1) as ctxp,
                tc.tile_pool(name=pfx + "tpd", bufs=4) as tpd,
                tc.tile_pool(name=pfx + "small", bufs=2) as small,
                tc.tile_pool(name=pfx + "ppsum", bufs=2, space="PSUM") as ppsum,
                tc.tile_pool(name=pfx + "epsum", bufs=2, space="PSUM") as epsum,
                tc.tile_pool(name=pfx + "psctx", bufs=2, space="PSUM") as psctx,
            ):
                bqT = cpool.tile([128, NG], F32, tag="bqT", name=pfx + "bqT")
                bkT = cpool.tile([128, NG], F32, tag="bkT", name=pfx + "bkT")
                bsTn = cpool.tile([64, 1], F32, tag="bsTn", name=pfx + "bsTn")
                bv_rep = cpool.tile([128, HL * DH], F32, tag="bv_rep", name=pfx + "bv_rep")
                warm = cpool.tile([128, QB], FP16, tag="warm", name=pfx + "warm")
                nc.scalar.dma_start(bqT[:], bq.rearrange("(g p) -> p g", p=128))
                nc.scalar.dma_start(bkT[:], bk.rearrange("(g p) -> p g", p=128))
                nc.scalar.dma_start(bsTn[:], bs.rearrange("(g p) -> p g", p=64))
                nc.scalar.dma_start(
                    bv_rep[:], bv[None, :].to_broadcast((128, HL * DH))
                )
                nc.vector.tensor_scalar_mul(bsTn[:], bsTn[:], -1.0)
                nc.gpsimd.memset(warm[:], 0.125)

                # keep the PE HAM window busy while stage A DMAs run, so
                # projections start at 2.4 GHz instead of 1.2
                for i in range(36):
                    wp = ppsum.tile([128, QB], F32, tag="psproj", name=f"{pfx}wp{i}")
                    nc.tensor.matmul(
                        wp[:], warm[:, 0:128], warm[:], start=True, stop=True,
                        skip_group_check=True,
                    )

                # transposed fp16 operands: [do, j, m] = X[m, 128j+do]
                hidT = qkv.tile([128, EJ, S], FP16, tag="hidT", name=pfx + "hidT")
                wqT = qkv.tile([128, EJ, HL * DH], FP16, tag="wqT", name=pfx + "wqT")
                wkT = qkv.tile([128, EJ, HL * DH], FP16, tag="wkT", name=pfx + "wkT")
                wvT = qkv.tile([128, EJ, HL * DH], FP16, tag="wvT", name=pfx + "wvT")
                wsT = qkv.tile([128, EJ, 64], FP16, tag="wsT", name=pfx + "wsT")

                QT = [qkv.tile([128, S], FP16, tag=f"QT{g}", name=f"{pfx}QT{g}") for g in range(NG)]
                KT = [qkv.tile([128, S], FP16, tag=f"KT{g}", name=f"{pfx}KT{g}") for g in range(NG)]
                VA = [qkv.tile([128, HL * 65], F32R, tag=f"VA{t}", name=f"{pfx}VA{t}") for t in range(ST)]
                sval = qkv.tile([64, S], FP16, tag="sval", name=pfx + "sval")
                srep = [qkv.tile([128, S], FP16, tag=f"srep{g}", name=f"{pfx}srep{g}") for g in range(NG)]

                for t in range(ST):
                    va3 = VA[t].bitcast(F32).rearrange("p (h x) -> p h x", h=HL)
                    nc.gpsimd.memset(va3[:, :, 64], 1.0)

                # ---- stage A: loads on the sync ring (wk first, wq/ws/wv
                # just-in-time between hid tiles), converts on DVE, XBAR
                # transposes on the scalar ring ----
                def load_w(wdram, wT, rows, h):
                    pr = min(128, rows)
                    wr = raw.tile([128, E], F32, tag="w_raw")
                    wb = hbf.tile([128, E], FP16, tag="w_bf")
                    nc.sync.dma_start(wr[0:pr, :], wdram[128 * h : 128 * h + pr, :])
                    nc.vector.tensor_copy(wb[0:pr, :], wr[0:pr, :])
                    nc.scalar.dma_start_transpose(
                        wT[:, :, pr * h : pr * (h + 1)], wb[0:pr, :]
                    )

                def load_hid(i):
                    hr = raw.tile([128, E], F32, tag="hid_raw")
                    hb = hbf.tile([128, E], FP16, tag="hid_bf")
                    nc.sync.dma_start(hr[:], hid[128 * i : 128 * (i + 1), :])
                    nc.vector.tensor_copy(hb[:], hr[:])
                    nc.scalar.dma_start_transpose(
                        hidT[:, :, 128 * i : 128 * (i + 1)], hb[:]
                    )

                load_w(wk, wkT, 256, 0)
                load_w(wk, wkT, 256, 1)
                for i in range(4):
                    load_hid(i)
                load_w(wq, wqT, 256, 0)
                load_w(wq, wqT, 256, 1)
                load_w(ws, wsT, 64, 0)
                for i in range(4, 8):
                    load_hid(i)
                load_w(wv, wvT, 256, 0)
                load_w(wv, wvT, 256, 1)
                for i in range(8, ST):
                    load_hid(i)

                def k_proj(qb):
                    sl = slice(QB * qb, QB * (qb + 1))
                    for g in range(NG):
                        psk = ppsum.tile([128, QB], F32, tag="psproj")
                        for j in range(EJ):
                            nc.tensor.matmul(
                                psk[:],
                                wkT[:, j, 128 * g : 128 * (g + 1)],
                                hidT[:, j, sl],
                                start=(j == 0),
                                stop=(j == EJ - 1),
                            )
                        nc.vector.tensor_scalar_add(
                            KT[g][:, sl], psk[:], bkT[:, g : g + 1]
                        )

                def s_proj(qb):
                    # sigmoid via the exp table (avoids ACT table reloads
                    # between the attention exps): s = 1/(1 + exp(-(x+bs)))
                    sl = slice(QB * qb, QB * (qb + 1))
                    pss = ppsum.tile([128, QB], F32, tag="psproj")
                    for j in range(EJ):
                        nc.tensor.matmul(
                            pss[0:64, :],
                            wsT[:, j, :],
                            hidT[:, j, sl],
                            start=(j == 0),
                            stop=(j == EJ - 1),
                        )
                    sexp = small.tile([64, QB], F32, tag="sexp")
                    nc.scalar.activation(
                        sexp[:], pss[0:64, :], AF.Exp, scale=-1.0, bias=bsTn[:, 0:1]
                    )
                    nc.vector.tensor_scalar_add(sexp[:], sexp[:], 1.0)
                    nc.vector.reciprocal(sexp[:], sexp[:])
                    nc.vector.tensor_scalar(
                        sval[:, sl], sexp[:], 0.1, 0.95, ALU.mult, ALU.add
                    )
                    for g in range(NG):
                        nc.sync.dma_start(
                            srep[g][:, sl],
                            sval[32 * g : 32 * (g + 1), sl][:, None, :]
                            .to_broadcast((32, 4, QB)),
                        )

                def q_proj(qb, g):
                    sl = slice(QB * qb, QB * (qb + 1))
                    psq = ppsum.tile([128, QB], F32, tag="psproj")
                    for j in range(EJ):
                        nc.tensor.matmul(
                            psq[:],
                            wqT[:, j, 128 * g : 128 * (g + 1)],
                            hidT[:, j, sl],
                            start=(j == 0),
                            stop=(j == EJ - 1),
                        )
                    nc.vector.scalar_tensor_tensor(
                        QT[g][:, sl],
                        psq[:],
                        bqT[:, g : g + 1],
                        srep[g][:, sl],
                        ALU.add,
                        ALU.mult,
                    )

                def v_proj(t):
                    psv = ppsum.tile([128, QB], F32, tag="psproj")
                    for j in range(EJ):
                        nc.tensor.matmul(
                            psv[:, 0 : HL * DH],
                            hidT[:, j, 128 * t : 128 * (t + 1)],
                            wvT[:, j, :],
                            start=(j == 0),
                            stop=(j == EJ - 1),
                        )
                    va3 = VA[t].rearrange("p (h x) -> p h x", h=HL)
                    nc.vector.tensor_tensor(
                        va3[:, :, 0:64],
                        psv[:, 0 : HL * DH].rearrange("p (h d) -> p h d", h=HL),
                        bv_rep.rearrange("p (h d) -> p h d", h=HL),
                        ALU.add,
                    )

                # K projection fully (scores(t) needs KT column block t//4)
                for qb in range(S // QB):
                    k_proj(qb)
                s_proj(0)
                q_proj(0, 0)
                q_proj(0, 1)

                if stages == "proj":
                    for qb in range(1, S // QB):
                        s_proj(qb)
                        q_proj(qb, 0)
                        q_proj(qb, 1)
                    for t in range(ST):
                        v_proj(t)
                    return

                cs_bufs = [
                    ctxp.tile([128, QB], FP16, tag=f"cs{i}", name=f"{pfx}cs{i}")
                    for i in range(4)
                ]
                for c in cs_bufs:
                    nc.gpsimd.memset(c[:], 0.0)

                # section s = (qblk, g); scores/exp(sec) interleaved with
                # ctx(sec-1) at t granularity to keep the PE HAM-warm
                all_ets = {}
                psc_live = {}
                outs_live = {}

                def scores_step(sec, t):
                    qblk, g = divmod(sec, NG)
                    qsl = slice(QB * qblk, QB * (qblk + 1))
                    pss = epsum.tile([128, 2 * QB], F32, tag="psbig")
                    for sub in range(2):
                        hb = 64 * sub
                        nc.tensor.matmul(
                            pss[:, QB * sub : QB * (sub + 1)],
                            KT[g][hb : hb + 64, 128 * t : 128 * (t + 1)],
                            QT[g][hb : hb + 64, qsl],
                            start=True,
                            stop=True,
                        )
                    et = expp.tile([128, 2 * QB], F32R, tag="expT")
                    nc.scalar.activation(et[:], pss[:], AF.Exp, scale=0.125)
                    all_ets[(sec, t)] = et

                def ctx_step(sec, t):
                    for sub in range(2):
                        _, g = divmod(sec, NG)
                        head = 2 * g + sub
                        if t == 0:
                            psc_live[(sec, sub)] = psctx.tile(
                                [65, QB], F32, tag="psc", name=f"{pfx}psc{sec}_{sub}"
                            )
                        nc.tensor.matmul(
                            psc_live[(sec, sub)],
                            VA[t][:, 65 * head : 65 * (head + 1)],
                            all_ets[(sec, t)][:, QB * sub : QB * (sub + 1)],
                            start=(t == 0),
                            stop=(t == ST - 1),
                        )

                def ctx_finish(sec):
                    qblk, g = divmod(sec, NG)
                    if g == 0:
                        outs_live[qblk] = outp.tile(
                            [128, 4, HL * DH], F32, tag="out_sb",
                            name=f"{pfx}out_sb_{qblk}",
                        )
                    outs = outs_live[qblk]
                    for sub in range(2):
                        head = 2 * g + sub
                        psc = psc_live.pop((sec, sub))
                        cs = cs_bufs[2 * g + sub]
                        nc.vector.tensor_copy(cs[0:65, :], psc[:])
                        pst = tpd.tile([128, 4, 128], FP16, tag="pst")
                        nc.sync.dma_start_transpose(pst[:], cs[:])
                        rec = small.tile([128, 4], F32, tag="rec")
                        nc.vector.reciprocal(
                            rec[:],
                            pst[:, :, 64:65].rearrange("p a b -> p (a b)"),
                        )
                        nc.vector.tensor_tensor(
                            outs[:, :, DH * head : DH * (head + 1)],
                            pst[:, :, 0:64],
                            rec[:, :, None].to_broadcast((128, 4, 64)),
                            ALU.mult,
                        )
                    if g == NG - 1:
                        qsl = slice(QB * qblk, QB * (qblk + 1))
                        nc.sync.dma_start(
                            out[qsl].rearrange("(qt p) c -> p qt c", p=128),
                            outs[:],
                        )

                for sec in range(NSEC):
                    qblk = sec // NG
                    for t in range(ST):
                        scores_step(sec, t)
                        if sec == 0:
                            v_proj(t)
                        elif sec == 1 and t < 6 and t % 2 == 0:
                            # spread remaining s/Q projections into sec 1
                            qb = 1 + t // 2
                            s_proj(qb)
                            q_proj(qb, 0)
                            q_proj(qb, 1)
                        if sec >= 1:
                            ctx_step(sec - 1, t)
                            if t == ST - 1:
                                ctx_finish(sec - 1)
                            # free consumed exp tiles promptly
                            del all_ets[(sec - 1, t)]
                # last section has no successor to interleave with
                for t in range(ST):
                    ctx_step(NSEC - 1, t)
                ctx_finish(NSEC - 1)

        for rep in range(reps):
            emit(f"R{rep}" if reps > 1 else "")
    return nc


_NC = None


def _get_compiled():
    global _NC
    if _NC is None:
        nc = bacc.Bacc(
            "TRN2",
            target_bir_lowering=False,
            debug=False,
            enable_asserts=False,
            num_devices=8,
        )
        build_program(nc)
        nc.compile()
        _NC = nc
    return _NC


def make_in_maps(hidden_states, Wq, bq, Wk, bk, Wv, bv, Ws, bs):
    c32 = lambda a: np.ascontiguousarray(a, dtype=np.float32)
    in_maps = []
    for c in range(8):
        b, hq = divmod(c, 4)
        r = slice(256 * hq, 256 * (hq + 1))
        rs = slice(64 * hq, 64 * (hq + 1))
        in_maps.append(
            {
                "hid": c32(hidden_states[b]),
                "wq": c32(Wq[r]), "bq": c32(bq[r]),
                "wk": c32(Wk[r]), "bk": c32(bk[r]),
                "wv": c32(Wv[r]), "bv": c32(bv[r]),
                "ws": c32(Ws[rs]), "bs": c32(bs[rs]),
            }
        )
    return in_maps


def assemble(results):
    out = np.empty((2, S, 1024), np.float32)
    for c in range(8):
        b, hq = divmod(c, 4)
        out[b, :, 256 * hq : 256 * (hq + 1)] = results[c]["out"]
    return out


def kernel(hidden_states, Wq, bq, Wk, bk, Wv, bv, Ws, bs):
    nc = _get_compiled()
    in_maps = make_in_maps(hidden_states, Wq, bq, Wk, bk, Wv, bv, Ws, bs)
    res = bass_utils.run_bass_kernel_spmd(nc, in_maps, core_ids=list(range(8)))
    return assemble(res.results)


# revision 24
# speedup vs baseline: 1.4191x; 1.4191x over previous
"""Trainium2 Bass kernel for CurvSelfAttention (B=2, S=2048, E=1024, H=16).

Sharding: 8 cores = 2 batches x 4 head-quads. Core c handles batch c//4 and
heads [4*(c%4), 4*(c%4)+4). Attention is head-independent, so there are no
collectives; each core gets its batch's hidden states and its heads' weight
row-slices, and returns a [S, 256] slice of the output.

v5 design, from on-silicon microbenchmarks and traces:
  - ScalarE exp is the hard floor: 128 x [128,1024] exp tiles at ~1.15us
    each (1.2 GHz, fixed). The schedule aims ScalarE at 100% exp duty.
  - exp OUTPUT dtype f32r is the fast ACT path (1115ns vs 1337 f32 /
    1403 bf16 / 1521 fp16), so ets and the ctx matmuls are f32r
    (32/16-bit matmul mixing is unsupported). The sigmoid for the group
    scales is computed with the exp table (1/(1+e^-x) via DVE) so the
    ACT activation table never reloads mid-stream.
  - fp16 matmuls run at full rate (~215ns/MM N=512) with near-exact
    products, so projections and scores use fp16 operands.
  - The DMA XBAR transpose measured ~6-8us per [128,1024] call end to
    end and serialized the preamble; hid/weight transposes instead run
    on the PE in fp16 (1 cyc/row, 8 transposes batched into one 1-bank
    fp16 psum tile, one strided DVE copy out per tile). This also keeps
    the PE HAM-warm through stage A. Only the small per-section ctx
    output transposes use the XBAR, off the critical path.
  - PE HAM re-throttles to 1.2 GHz after ~3.4us idle; the attention loop
    interleaves scores(sec, t) + ctx(sec-1, t) at t-tile granularity and
    spreads V/s/Q projection work into the per-step slack of the early
    sections. hid loads are split across both hwdge rings.
"""

import numpy as np

import concourse.bass as bass
import concourse.mybir as mybir
import concourse.tile as tile
from concourse import bacc, bass_utils
from concourse.masks import make_identity

S = 2048
E = 1024
HL = 4          # heads per core
DH = 64         # head dim
NG = 2          # head groups per core (2 heads each -> 128 partitions)
EJ = E // 128   # 8 contraction tiles
ST = S // 128   # 16 sequence tiles
QB = 512        # projection / attention q block
NSEC = (S // QB) * NG
F32 = mybir.dt.float32
F32R = mybir.dt.float32r
FP16 = mybir.dt.float16


def build_program(nc, reps=1, stages="all"):
    hid = nc.dram_tensor("hid", [S, E], F32, kind="ExternalInput")
    wq = nc.dram_tensor("wq", [HL * DH, E], F32, kind="ExternalInput")
    wk = nc.dram_tensor("wk", [HL * DH, E], F32, kind="ExternalInput")
    wv = nc.dram_tensor("wv", [HL * DH, E], F32, kind="ExternalInput")
    ws = nc.dram_tensor("ws", [64, E], F32, kind="ExternalInput")
    bq = nc.dram_tensor("bq", [HL * DH], F32, kind="ExternalInput")
    bk = nc.dram_tensor("bk", [HL * DH], F32, kind="ExternalInput")
    bv = nc.dram_tensor("bv", [HL * DH], F32, kind="ExternalInput")
    bs = nc.dram_tensor("bs", [64], F32, kind="ExternalInput")
    out = nc.dram_tensor("out", [S, HL * DH], F32, kind="ExternalOutput")

    AF = mybir.ActivationFunctionType
    ALU = mybir.AluOpType

    with tile.TileContext(nc) as tc:
        def emit(pfx):
            with (
                tc.tile_pool(name=pfx + "const", bufs=1) as cpool,
                tc.tile_pool(name=pfx + "qkv", bufs=1) as qkv,
                tc.tile_pool(name=pfx + "raw", bufs=3) as raw,
                tc.tile_pool(name=pfx + "hbf", bufs=3) as hbf,
                tc.tile_pool(name=pfx + "outp", bufs=2) as outp,
                tc.tile_pool(name=pfx + "expT", bufs=17) as expp,
                tc.tile_pool(name=pfx + "ctxsb", bufs=1) as ctxp,
                tc.tile_pool(name=pfx + "tpd", bufs=4) as tpd,
                tc.tile_pool(name=pfx + "small", bufs=2) as small,
                tc.tile_pool(name=pfx + "ppsum", bufs=2, space="PSUM") as ppsum,
            ):
                bqT = cpool.tile([128, NG], F32, tag="bqT", name=pfx + "bqT")
                bkT = cpool.tile([128, NG], F32, tag="bkT", name=pfx + "bkT")
                bsTn = cpool.tile([64, 1], F32, tag="bsTn", name=pfx + "bsTn")
                bv_rep = cpool.tile([128, HL * DH], F32, tag="bv_rep", name=pfx + "bv_rep")
                ident = cpool.tile([128, 128], FP16, tag="ident", name=pfx + "ident")
                nc.scalar.dma_start(bqT[:], bq.rearrange("(g p) -> p g", p=128))
                nc.scalar.dma_start(bkT[:], bk.rearrange("(g p) -> p g", p=128))
                nc.scalar.dma_start(bsTn[:], bs.rearrange("(g p) -> p g", p=64))
                nc.scalar.dma_start(
                    bv_rep[:], bv[None, :].to_broadcast((128, HL * DH))
                )
                nc.vector.tensor_scalar_mul(bsTn[:], bsTn[:], -1.0)
                make_identity(nc, ident[:])

                # transposed fp16 operands: [do, j, m] = X[m, 128j+do]
                hidT = qkv.tile([128, EJ, S], FP16, tag="hidT", name=pfx + "hidT")
                wqT = qkv.tile([128, EJ, HL * DH], FP16, tag="wqT", name=pfx + "wqT")
                wkT = qkv.tile([128, EJ, HL * DH], FP16, tag="wkT", name=pfx + "wkT")
                wvT = qkv.tile([128, EJ, HL * DH], FP16, tag="wvT", name=pfx + "wvT")
                wsT = qkv.tile([128, EJ, 64], FP16, tag="wsT", name=pfx + "wsT")

                QT = [qkv.tile([128, S], FP16, tag=f"QT{g}", name=f"{pfx}QT{g}") for g in range(NG)]
                KT = [qkv.tile([128, S], FP16, tag=f"KT{g}", name=f"{pfx}KT{g}") for g in range(NG)]
                VA = [qkv.tile([128, HL * 65], F32R, tag=f"VA{t}", name=f"{pfx}VA{t}") for t in range(ST)]
                sval = qkv.tile([64, S], FP16, tag="sval", name=pfx + "sval")
                srep = [qkv.tile([128, S], FP16, tag=f"srep{g}", name=f"{pfx}srep{g}") for g in range(NG)]

                for t in range(ST):
                    va3 = VA[t].bitcast(F32).rearrange("p (h x) -> p h x", h=HL)
                    nc.gpsimd.memset(va3[:, :, 64], 1.0)

                # ---- PE transposes (fp16, 8 per psum bank, 1 DVE copy) ----
                def transpose_tile(src, dst3, cols, pstr):
                    # src [rows, 1024] -> dst3[:, j, 0:cols] = src[c, 128j+do]
                    ps = pstr.tile([128, EJ * 128], FP16, tag="pstr")
                    for j in range(EJ):
                        nc.tensor.transpose(
                            ps[:, 128 * j : 128 * j + cols],
                            src[0:cols, 128 * j : 128 * (j + 1)],
                            ident[0:cols, 0:cols],
                        )
                    nc.vector.tensor_copy(
                        dst3[:, :, 0:cols],
                        ps.rearrange("p (j c) -> p j c", j=EJ)[:, :, 0:cols],
                    )

                def k_proj(qb):
                    sl = slice(QB * qb, QB * (qb + 1))
                    for g in range(NG):
                        psk = ppsum.tile([128, QB], F32, tag="psproj")
                        for j in range(EJ):
                            nc.tensor.matmul(
                                psk[:],
                                wkT[:, j, 128 * g : 128 * (g + 1)],
                                hidT[:, j, sl],
                                start=(j == 0),
                                stop=(j == EJ - 1),
                            )
                        nc.vector.tensor_scalar_add(
                            KT[g][:, sl], psk[:], bkT[:, g : g + 1]
                        )

                def s_proj(qb):
                    # sigmoid via the exp table: s = 1/(1 + exp(-(x+bs)))
                    sl = slice(QB * qb, QB * (qb + 1))
                    pss = ppsum.tile([128, QB], F32, tag="psproj")
                    for j in range(EJ):
                        nc.tensor.matmul(
                            pss[0:64, :],
                            wsT[:, j, :],
                            hidT[:, j, sl],
                            start=(j == 0),
                            stop=(j == EJ - 1),
                        )
                    sexp = small.tile([64, QB], F32, tag="sexp")
                    nc.scalar.activation(
                        sexp[:], pss[0:64, :], AF.Exp, scale=-1.0, bias=bsTn[:, 0:1]
                    )
                    nc.vector.tensor_scalar_add(sexp[:], sexp[:], 1.0)
                    nc.vector.reciprocal(sexp[:], sexp[:])
                    nc.vector.tensor_scalar(
                        sval[:, sl], sexp[:], 0.1, 0.95, ALU.mult, ALU.add
                    )
                    for g in range(NG):
                        nc.sync.dma_start(
                            srep[g][:, sl],
                            sval[32 * g : 32 * (g + 1), sl][:, None, :]
                            .to_broadcast((32, 4, QB)),
                        )

                def q_proj(qb, g):
                    sl = slice(QB * qb, QB * (qb + 1))
                    psq = ppsum.tile([128, QB], F32, tag="psproj")
                    for j in range(EJ):
                        nc.tensor.matmul(
                            psq[:],
                            wqT[:, j, 128 * g : 128 * (g + 1)],
                            hidT[:, j, sl],
                            start=(j == 0),
                            stop=(j == EJ - 1),
                        )
                    nc.vector.scalar_tensor_tensor(
                        QT[g][:, sl],
                        psq[:],
                        bqT[:, g : g + 1],
                        srep[g][:, sl],
                        ALU.add,
                        ALU.mult,
                    )

                def v_proj(t):
                    psv = ppsum.tile([128, QB], F32, tag="psproj")
                    for j in range(EJ):
                        nc.tensor.matmul(
                            psv[:, 0 : HL * DH],
                            hidT[:, j, 128 * t : 128 * (t + 1)],
                            wvT[:, j, :],
                            start=(j == 0),
                            stop=(j == EJ - 1),
                        )
                    va3 = VA[t].rearrange("p (h x) -> p h x", h=HL)
                    nc.vector.tensor_tensor(
                        va3[:, :, 0:64],
                        psv[:, 0 : HL * DH].rearrange("p (h d) -> p h d", h=HL),
                        bv_rep.rearrange("p (h d) -> p h d", h=HL),
                        ALU.add,
                    )

                # stage A: fused load+convert+transpose per tile, emitted in
                # strict consumption order (pool rotation WAR deps stay
                # acyclic); hid alternates between both hwdge rings
                with tc.tile_pool(name=pfx + "ptr", bufs=2, space="PSUM") as pstr:
                    def lt(dram, rows, ring, dst3, cols):
                        wr = raw.tile([128, E], F32, tag="w_raw")
                        wb = hbf.tile([128, E], FP16, tag="w_bf")
                        ring.dma_start(wr[0:rows, :], dram)
                        nc.vector.tensor_copy(wb[0:rows, :], wr[0:rows, :])
                        transpose_tile(wb, dst3, cols, pstr)

                    def lt_hid(i):
                        ring = nc.sync if i % 2 == 0 else nc.scalar
                        lt(hid[128 * i : 128 * (i + 1), :], 128, ring,
                           hidT[:, :, 128 * i : 128 * (i + 1)], 128)

                    def lt_w(dram, wT, h):
                        lt(dram[128 * h : 128 * (h + 1), :], 128, nc.scalar,
                           wT[:, :, 128 * h : 128 * (h + 1)], 128)

                    lt(wk[0:128, :], 128, nc.sync, wkT[:, :, 0:128], 128)
                    lt(wk[128:256, :], 128, nc.sync, wkT[:, :, 128:256], 128)
                    for i in range(4):
                        lt_hid(i)
                    k_proj(0)
                    lt_w(wq, wqT, 0)
                    lt_w(wq, wqT, 1)
                    lt(ws[0:64, :], 64, nc.scalar, wsT[:, :, 0:64], 64)
                    for i in range(4, 8):
                        lt_hid(i)
                    k_proj(1)
                    s_proj(0)
                    for i in range(8, 12):
                        lt_hid(i)
                    k_proj(2)
                    q_proj(0, 0)
                    q_proj(0, 1)
                    for i in range(12, ST):
                        lt_hid(i)
                    k_proj(3)
                    lt_w(wv, wvT, 0)
                    lt_w(wv, wvT, 1)

                if stages == "proj":
                    for qb in range(1, S // QB):
                        s_proj(qb)
                        q_proj(qb, 0)
                        q_proj(qb, 1)
                    for t in range(ST):
                        v_proj(t)
                    return

                # per-section filler work (one item per t-step, from step 4)
                fillers = {
                    0: [lambda t=t: v_proj(t) for t in range(10)],
                    1: ([lambda t=t: v_proj(t) for t in range(10, ST)]
                        + [lambda: s_proj(1), lambda: q_proj(1, 0), lambda: q_proj(1, 1)]),
                    2: [lambda: s_proj(2), lambda: q_proj(2, 0), lambda: q_proj(2, 1)],
                    3: [lambda: s_proj(3), lambda: q_proj(3, 0), lambda: q_proj(3, 1)],
                }

                cs_bufs = [
                    ctxp.tile([128, QB], FP16, tag=f"cs{i}", name=f"{pfx}cs{i}")
                    for i in range(4)
                ]
                for c in cs_bufs:
                    nc.gpsimd.memset(c[:], 0.0)

                with (
                    tc.tile_pool(name=pfx + "epsum", bufs=2, space="PSUM") as epsum,
                    tc.tile_pool(name=pfx + "psctx", bufs=2, space="PSUM") as psctx,
                ):
                    all_ets = {}
                    psc_live = {}
                    outs_live = {}

                    def scores_step(sec, t):
                        qblk, g = divmod(sec, NG)
                        qsl = slice(QB * qblk, QB * (qblk + 1))
                        pss = epsum.tile([128, 2 * QB], F32, tag="psbig")
                        for sub in range(2):
                            hb = 64 * sub
                            nc.tensor.matmul(
                                pss[:, QB * sub : QB * (sub + 1)],
                                KT[g][hb : hb + 64, 128 * t : 128 * (t + 1)],
                                QT[g][hb : hb + 64, qsl],
                                start=True,
                                stop=True,
                            )
                        et = expp.tile([128, 2 * QB], F32R, tag="expT")
                        nc.scalar.activation(et[:], pss[:], AF.Exp, scale=0.125)
                        all_ets[(sec, t)] = et

                    def ctx_step(sec, t):
                        for sub in range(2):
                            _, g = divmod(sec, NG)
                            head = 2 * g + sub
                            if t == 0:
                                psc_live[(sec, sub)] = psctx.tile(
                                    [65, QB], F32, tag="psc",
                                    name=f"{pfx}psc{sec}_{sub}",
                                )
                            nc.tensor.matmul(
                                psc_live[(sec, sub)],
                                VA[t][:, 65 * head : 65 * (head + 1)],
                                all_ets[(sec, t)][:, QB * sub : QB * (sub + 1)],
                                start=(t == 0),
                                stop=(t == ST - 1),
                            )

                    def ctx_finish(sec):
                        qblk, g = divmod(sec, NG)
                        if g == 0:
                            outs_live[qblk] = outp.tile(
                                [128, 4, HL * DH], F32, tag="out_sb",
                                name=f"{pfx}out_sb_{qblk}",
                            )
                        outs = outs_live[qblk]
                        for sub in range(2):
                            head = 2 * g + sub
                            psc = psc_live.pop((sec, sub))
                            cs = cs_bufs[2 * g + sub]
                            nc.vector.tensor_copy(cs[0:65, :], psc[:])
                            pst = tpd.tile([128, 4, 128], FP16, tag="pst")
                            nc.sync.dma_start_transpose(pst[:], cs[:])
                            rec = small.tile([128, 4], F32, tag="rec")
                            nc.vector.reciprocal(
                                rec[:],
                                pst[:, :, 64:65].rearrange("p a b -> p (a b)"),
                            )
                            nc.vector.tensor_tensor(
                                outs[:, :, DH * head : DH * (head + 1)],
                                pst[:, :, 0:64],
                                rec[:, :, None].to_broadcast((128, 4, 64)),
                                ALU.mult,
                            )
                        if g == NG - 1:
                            qsl = slice(QB * qblk, QB * (qblk + 1))
                            nc.sync.dma_start(
                                out[qsl].rearrange("(qt p) c -> p qt c", p=128),
                                outs[:],
                            )

                    for sec in range(NSEC):
                        fl = fillers.get(sec, [])
                        fi = 0
                        for t in range(ST):
                            scores_step(sec, t)
                            if t >= 4 and fi < len(fl):
                                fl[fi]()
                                fi += 1
                            if sec >= 1:
                                ctx_step(sec - 1, t)
                                if t == ST - 1:
                                    ctx_finish(sec - 1)
                                del all_ets[(sec - 1, t)]
                        while fi < len(fl):
                            fl[fi]()
                            fi += 1
                    for t in range(ST):
                        ctx_step(NSEC - 1, t)
                    ctx_finish(NSEC - 1)

        for rep in range(reps):
            emit(f"R{rep}" if reps > 1 else "")
    return nc


_NC = None


def _get_compiled():
    global _NC
    if _NC is None:
        nc = bacc.Bacc(
            "TRN2",
            target_bir_lowering=False,
            debug=False,
            enable_asserts=False,
            num_devices=8,
        )
        build_program(nc)
        nc.compile()
        _NC = nc
    return _NC


def make_in_maps(hidden_states, Wq, bq, Wk, bk, Wv, bv, Ws, bs):
    c32 = lambda a: np.ascontiguousarray(a, dtype=np.float32)
    in_maps = []
    for c in range(8):
        b, hq = divmod(c, 4)
        r = slice(256 * hq, 256 * (hq + 1))
        rs = slice(64 * hq, 64 * (hq + 1))
        in_maps.append(
            {
                "hid": c32(hidden_states[b]),
                "wq": c32(Wq[r]), "bq": c32(bq[r]),
                "wk": c32(Wk[r]), "bk": c32(bk[r]),
                "wv": c32(Wv[r]), "bv": c32(bv[r]),
                "ws": c32(Ws[rs]), "bs": c32(bs[rs]),
            }
        )
    return in_maps


def assemble(results):
    out = np.empty((2, S, 1024), np.float32)
    for c in range(8):
        b, hq = divmod(c, 4)
        out[b, :, 256 * hq : 256 * (hq + 1)] = results[c]["out"]
    return out


def kernel(hidden_states, Wq, bq, Wk, bk, Wv, bv, Ws, bs):
    nc = _get_compiled()
    in_maps = make_in_maps(hidden_states, Wq, bq, Wk, bk, Wv, bv, Ws, bs)
    res = bass_utils.run_bass_kernel_spmd(nc, in_maps, core_ids=list(range(8)))
    return assemble(res.results)
